# revision 38
# baseline (speedup 1.0000x reference)
"""Trainium2 Bass kernel for nn_Cross_Domain_Class_Alignment.

Reference computation (per sample b):
    mask0[b] = argmin_k || feature_s2t[b,:,r,c] - centroid_target[k] ||^2
    mask1[b] = argmin_k || feature_target[b,:,r,c] - centroid_s2t[k] ||^2
    both nearest-upsampled from (65,129) to (512,1024), int32.

Sharding: data-parallel over batch B=8 across 8 NeuronCores (1 sample/core).
Centroids are replicated.

Per-core dataflow (per mask):
  - features [256, 8385] streamed in 2048-pixel slices x 2 channel chunks;
    the 193-px remainder is processed FIRST so the pipeline tail is short
  - dist matmuls, centroid-stationary: psum quad [128, 512] holds four
    512-pixel banks stacked at partition offsets {0,32,64,96} via
    tile_position col-tiling.  Two chunk matmuls accumulate C=256.
  - scalar-engine copy fuses m = 2*dots - csq (per-partition bias) while
    moving the quad PSUM->SBUF
  - ONE DVE stream-transpose (32x32 blocks) per quad flips classes onto
    columns; the DVE argmin (reduce_max / is_ge / *(19-k) / reduce_max,
    first-index tie-break) then reduces within blocks; the {0,1}/weight
    tail runs in bf16 (exact).  This keeps the PE off the critical path
    (fp32 PE transposes run at 2 cycles/row and were the tail bottleneck).
  - per quad: PE transpose of ptf [128,16] -> scalar ACT fuses
    idx = 19 - y and casts to int8 -> 8KB DRAM bounce (block-strided
    scratch AP) reshapes the flat pixel order into m [65, 129] int8
  - column nearest-upsample 129 -> 1024 in THREE DVE copies via the affine
    decomposition out[127a + 8c + k] = m[16a + c] (+ two edge arms)
  - row nearest-upsample is folded into the output DMAs: 16 row-segment
    stores with stride-0 (broadcast) source APs replicate each source row
    7-8x on the way to DRAM; output is int8 (cast to int32 on host)
"""

import numpy as np

B, C, h, w = 8, 256, 65, 129
K = 19
H, W = 512, 1024
HW = h * w              # 8385
QUAD_PX = 2048          # four 512-px banks per psum quad
NFULL = HW // QUAD_PX   # 4 full quads
REM = HW - NFULL * QUAD_PX   # 193 remainder pixels
NT = (HW + 127) // 128  # 66 pixel blocks of 128


def _row_segs():
    """Runs of equal repeat count in the row map ri[r'] = r'*65 // 512."""
    ri = (np.arange(H) * h) // H
    rreps = np.bincount(ri, minlength=h)
    segs, i, dst = [], 0, 0
    while i < h:
        j = i
        while j < h and rreps[j] == rreps[i]:
            j += 1
        segs.append((i, j - i, int(rreps[i]), dst))
        dst += (j - i) * int(rreps[i])
        i = j
    assert dst == H
    return segs


# stripe split at row 32: DVE access patterns must start at a partition
# multiple of 32, so the colexp stripes are [0,32) and [32,65)
ROW_SEGS = _row_segs()
SEGS_A = [s for s in ROW_SEGS if s[0] + s[1] - 1 <= 31]   # rows 0..31
SEGS_B = [s for s in ROW_SEGS if s[0] + s[1] - 1 > 31]    # rows 32..64


def build_module(num_devices=8):
    import concourse.bass as bass
    import concourse.tile as tile
    from concourse import bacc, mybir
    import ml_dtypes

    f32 = mybir.dt.float32
    f32r = mybir.dt.float32r
    bf16 = mybir.dt.bfloat16
    i8 = mybir.dt.int8

    nc = bacc.Bacc(
        "TRN2",
        target_bir_lowering=False,
        debug=False,
        enable_asserts=False,
        num_devices=num_devices,
    )

    f_s2t = nc.dram_tensor("feature_s2t", [C, HW], f32, kind="ExternalInput")
    f_tgt = nc.dram_tensor("feature_target", [C, HW], f32, kind="ExternalInput")
    c_s2t = nc.dram_tensor("centroid_s2t", [K, C], f32, kind="ExternalInput")
    c_tgt = nc.dram_tensor("centroid_target", [K, C], f32, kind="ExternalInput")
    out0 = nc.dram_tensor("out0", [H, W], i8, kind="ExternalOutput")
    out1 = nc.dram_tensor("out1", [H, W], i8, kind="ExternalOutput")

    ident_dram = nc.inline_tensor(np.eye(128, dtype=np.float32), name="ident_const")
    # sel[k, 32j+k] = -1.0: replicates -csq over the four 32-partition groups
    sel_np = np.zeros((K, 128), dtype=np.float32)
    for j in range(4):
        sel_np[np.arange(K), 32 * j + np.arange(K)] = -1.0
    sel_dram = nc.inline_tensor(sel_np, name="sel_const")
    wk_np = np.tile(
        (K - np.arange(K)).astype(ml_dtypes.bfloat16), (128, 1)
    )
    wk_dram = nc.inline_tensor(wk_np, name="wk_const")

    X = mybir.AxisListType.X
    ALU = mybir.AluOpType
    AF = mybir.ActivationFunctionType

    def strided(ap, off, dims):
        """Replace the free dims of `ap` with explicit (stride, size) pairs."""
        a = ap.copy()
        a.ap = a.ap[0:1] + [[s, n] for (s, n) in dims]
        a.offset = a.offset + off
        return a

    def dram_strided(ap, off, dims):
        """Replace ALL dims of a DRAM `ap` with explicit (stride, size) pairs."""
        a = ap.copy()
        a.ap = a.ap[0:0] + [[s, n] for (s, n) in dims]
        a.offset = a.offset + off
        return a

    with tile.TileContext(nc) as tc:
        from contextlib import ExitStack

        with ExitStack() as ctx:
            const_p = ctx.enter_context(tc.tile_pool(name="const", bufs=1))
            feat_p = ctx.enter_context(tc.tile_pool(name="feat", bufs=7))
            q_p = ctx.enter_context(tc.tile_pool(name="q", bufs=4))
            qt_p = ctx.enter_context(tc.tile_pool(name="qt", bufs=4))
            mx_p = ctx.enter_context(tc.tile_pool(name="mx", bufs=3))
            eq_p = ctx.enter_context(tc.tile_pool(name="eq", bufs=3))
            pt_p = ctx.enter_context(tc.tile_pool(name="pt", bufs=3))
            m_p = ctx.enter_context(tc.tile_pool(name="m", bufs=2))
            ps_dist = ctx.enter_context(tc.tile_pool(name="psd", bufs=4, space="PSUM"))
            ps_tr = ctx.enter_context(tc.tile_pool(name="pst", bufs=3, space="PSUM"))
            dram_p = ctx.enter_context(tc.tile_pool(name="dram", bufs=2, space="DRAM"))

            # ---- constants (centroids first: they gate the prep chain) ----
            cent_sbs = {}
            for pidx, cdram in ((0, c_tgt), (1, c_s2t)):
                cs = const_p.tile(
                    [K, C], f32, tag=f"cent{pidx}", name=f"cent_sb{pidx}"
                )
                nc.scalar.dma_start(out=cs[:], in_=cdram[:, :])
                cent_sbs[pidx] = cs
            ident = const_p.tile([128, 128], f32, tag="ident")
            nc.scalar.dma_start(out=ident[:], in_=ident_dram[:, :])
            wk_sb = const_p.tile([128, K], bf16, tag="wk")
            nc.scalar.dma_start(out=wk_sb[:], in_=wk_dram[:, :])
            sel_sb = const_p.tile([K, 128], f32, tag="sel")
            nc.scalar.dma_start(out=sel_sb[:], in_=sel_dram[:, :])

            # ---- per-pair centroid prep ----
            def prep_pair(pidx):
                cent_sb = cent_sbs[pidx]
                sq = const_p.tile([K, C], f32, tag=f"centsq{pidx}")
                nc.vector.tensor_mul(sq[:], cent_sb[:], cent_sb[:])
                csq = const_p.tile([K, 1], f32, tag=f"csq{pidx}")
                nc.vector.reduce_sum(csq[:], sq[:], axis=X)
                # -csq replicated at partition offsets {0,32,64,96}
                pb = ps_tr.tile([128, 1], f32, tag="tr")
                nc.tensor.matmul(pb[:], sel_sb[:], csq[:], start=True, stop=True)
                csqn4 = const_p.tile([128, 1], f32, tag=f"csqn4_{pidx}")
                nc.vector.tensor_copy(out=csqn4[:], in_=pb[:])
                # centT chunks [128, 32]: cols 0:19 = cent^T, cols 19:32 = 0
                centT = []
                for cc in range(2):
                    ct = const_p.tile([128, 32], f32, tag=f"centT{pidx}_{cc}")
                    nc.vector.memset(ct[:], 0.0)
                    pt = ps_tr.tile([128, K], f32, tag="tr")
                    nc.tensor.transpose(
                        pt[:], cent_sb[:, cc * 128 : (cc + 1) * 128], ident[:K, :K]
                    )
                    nc.vector.tensor_copy(out=ct[:, 0:K], in_=pt[:])
                    centT.append(ct)
                return centT, csqn4

            centT_tgt, csqn4_tgt = prep_pair(0)   # for mask0 (feature_s2t)
            centT_s2t, csqn4_s2t = prep_pair(1)   # for mask1 (feature_target)

            def stream_mask(feat, centT, csqn4, out_dram, tail_engines):
                # ptf: quad Bq at cols 16*Bq+n (partition 32j+c''), rem at
                # cols 64..72 (partitions 0..31); px = 2048*Bq + 512j + 32n + c''
                ptf = pt_p.tile([128, 72], f32, tag="ptf")
                msb = m_p.tile([h, w], i8, tag="m")
                e_sb = m_p.tile([h, W], i8, tag="e")
                scratch = dram_p.tile([NT, 128], i8, tag="scratch")
                scratch_flat = scratch[:].rearrange("a b -> (a b)")

                def load_range(px0, pxw):
                    fg = []
                    for cc in range(2):
                        ft = feat_p.tile([128, QUAD_PX], f32, tag=f"feat{cc}")
                        half = (pxw + 1) // 2
                        nc.sync.dma_start(
                            out=ft[:, 0:half],
                            in_=feat[cc * 128 : (cc + 1) * 128, px0 : px0 + half],
                        )
                        nc.sync.dma_start(
                            out=ft[:, half:pxw],
                            in_=feat[cc * 128 : (cc + 1) * 128, px0 + half : px0 + pxw],
                        )
                        fg.append(ft)
                    return fg

                def argmin_g(src, npart, g, out_ap):
                    # y = 19 - argmin over k, first-index tie-break; src is
                    # the stream-transposed quad, class k at col 32*n + k
                    view = strided(src, 0, [(32, g), (1, K)])
                    mxt = mx_p.tile([128, g], f32, tag="mx")
                    nc.vector.tensor_reduce(mxt[0:npart, :], view, axis=X, op=ALU.max)
                    eqt = eq_p.tile([128, g * K], bf16, tag="eq")
                    eqv = eqt[0:npart, :].rearrange("p (g k) -> p g k", k=K)
                    nc.vector.tensor_tensor(
                        out=eqv,
                        in0=view,
                        in1=mxt[0:npart, :].unsqueeze(2).broadcast_to([npart, g, K]),
                        op=ALU.is_ge,
                    )
                    nc.vector.tensor_tensor(
                        out=eqv,
                        in0=eqv,
                        in1=wk_sb[0:npart, :].unsqueeze(1).broadcast_to([npart, g, K]),
                        op=ALU.mult,
                    )
                    nc.vector.tensor_reduce(out_ap, eqv, axis=X, op=ALU.max)

                def do_quad(Bq):
                    fg = load_range(Bq * QUAD_PX, QUAD_PX)
                    psq = ps_dist.tile([128, 512], f32, tag="dist")
                    for j in range(4):
                        for cc in range(2):
                            nc.tensor.matmul(
                                psq[32 * j : 32 * j + 32, :],
                                centT[cc][:],
                                fg[cc][:, 512 * j : 512 * j + 512],
                                start=(cc == 0),
                                stop=(cc == 1),
                                tile_position=(0, 32 * j),
                            )
                    quad = q_p.tile([128, 512], f32, tag="quad")
                    nc.scalar.activation(
                        out=quad[:],
                        in_=psq[:],
                        func=AF.Identity,
                        bias=csqn4[:],
                        scale=2.0,
                    )
                    # 32x32 block transpose: qT[32j+c'', 32n+k''] =
                    # value(class k'', px 2048*Bq + 512j + 32n + c'')
                    qT = qt_p.tile([128, 512], f32, tag="qt")
                    nc.vector.transpose(qT[:], quad[:])
                    argmin_g(qT[:], 128, 16, ptf[:, 16 * Bq : 16 * Bq + 16])

                def do_rem():
                    # remainder: 193 px (blocks 64, 65); done FIRST
                    px0 = NFULL * QUAD_PX
                    pxw = HW - px0
                    fg = load_range(px0, pxw)
                    psr = ps_dist.tile([32, 256], f32, tag="dist")
                    nc.vector.memset(psr[:, pxw:256], 0.0)
                    for cc in range(2):
                        nc.tensor.matmul(
                            psr[0:32, 0:pxw],
                            centT[cc][:],
                            fg[cc][:, 0:pxw],
                            start=(cc == 0),
                            stop=(cc == 1),
                        )
                    st2 = q_p.tile([32, 256], f32, tag="quad")
                    nc.scalar.activation(
                        out=st2[:],
                        in_=psr[:],
                        func=AF.Identity,
                        bias=csqn4[0:32, :],
                        scale=2.0,
                    )
                    qT = qt_p.tile([32, 256], f32, tag="qt")
                    nc.vector.transpose(qT[:], st2[:])
                    argmin_g(qT[:], 32, 8, ptf[0:32, 64:72])

                def ptt_quad(Bq):
                    # ptf quad cols -> scratch in flat pixel order, int8
                    pstt = ps_tr.tile([16, 128], f32, tag="tr")
                    nc.tensor.transpose(
                        pstt[:], ptf[:, 16 * Bq : 16 * Bq + 16], ident[:]
                    )
                    pttsb = pt_p.tile([16, 128], i8, tag="pttsb")
                    nc.scalar.activation(
                        out=pttsb[:],
                        in_=pstt[:],
                        func=AF.Copy,
                        bias=float(K),
                        scale=-1.0,
                    )
                    # scratch[2048*Bq + 512j + 32n + c''] <- pttsb[n, 32j+c'']
                    nc.scalar.dma_start(
                        out=dram_strided(
                            scratch_flat, 2048 * Bq, [(32, 16), (512, 4), (1, 32)]
                        ),
                        in_=strided(pttsb[:], 0, [(32, 4), (1, 32)]),
                    )

                def ptt_rem():
                    pstt = ps_tr.tile([8, 32], f32, tag="tr")
                    nc.tensor.transpose(
                        pstt[:], ptf[0:32, 64:72], ident[:32, :32]
                    )
                    pttsb = pt_p.tile([8, 32], i8, tag="pttsb")
                    nc.scalar.activation(
                        out=pttsb[:],
                        in_=pstt[:],
                        func=AF.Copy,
                        bias=float(K),
                        scale=-1.0,
                    )
                    # scratch[8192 + 32n + c''] <- pttsb[n, c'']
                    nc.scalar.dma_start(
                        out=dram_strided(scratch_flat, 8192, [(32, 8), (1, 32)]),
                        in_=pttsb[:],
                    )

                def bounce(r0, r1, eng=None):
                    (eng or nc.gpsimd).dma_start(
                        out=msb[r0:r1, :],
                        in_=scratch_flat[r0 * w : r1 * w].rearrange(
                            "(r c) -> r c", c=w
                        ),
                    )

                def colexp(r0, r1):
                    # column upsample 129 -> 1024: affine 3-arm decomposition
                    nr = r1 - r0
                    nc.vector.tensor_copy(
                        out=e_sb[r0:r1, 0:8],
                        in_=msb[r0:r1, 0:1].broadcast_to([nr, 8]),
                    )
                    nc.vector.tensor_copy(
                        out=strided(e_sb[r0:r1, :], 8, [(127, 8), (8, 15), (1, 8)]),
                        in_=strided(msb[r0:r1, :], 1, [(16, 8), (1, 15), (0, 8)]),
                    )
                    nc.vector.tensor_copy(
                        out=strided(e_sb[r0:r1, :], 128, [(127, 8), (1, 7)]),
                        in_=strided(msb[r0:r1, :], 16, [(16, 8), (0, 7)]),
                    )

                def store_seg(seg, eng):
                    src0, ns, rep, dst0 = seg
                    eng.dma_start(
                        out=out_dram[dst0 : dst0 + ns * rep, :].rearrange(
                            "(s r) c -> s r c", r=rep
                        ),
                        in_=e_sb[src0 : src0 + ns, :]
                        .unsqueeze(1)
                        .broadcast_to([ns, rep, W]),
                    )

                return {
                    "rem": do_rem,
                    "quad": do_quad,
                    "ptt_quad": ptt_quad,
                    "ptt_rem": ptt_rem,
                    "bounce": bounce,
                    "colexp": colexp,
                    "store": store_seg,
                }

            # The two masks are interleaved: each ptt transpose is emitted
            # ~two stream pieces behind its argmin so the in-order PE ring
            # never stalls on the DVE chain, and mask0's finish work rides
            # inside mask1's stream.
            m0 = stream_mask(f_s2t, centT_tgt, csqn4_tgt, out0, None)
            m1 = stream_mask(f_tgt, centT_s2t, csqn4_s2t, out1, None)

            m0["rem"]()
            m0["quad"](0)
            m0["quad"](1)
            m0["ptt_rem"]()
            m0["quad"](2)
            m0["ptt_quad"](0)
            m0["bounce"](0, 15)
            m0["quad"](3)
            m0["ptt_quad"](1)
            m0["bounce"](15, 31)
            m1["rem"]()
            m0["ptt_quad"](2)
            m0["bounce"](31, 47)
            m1["quad"](0)
            m0["ptt_quad"](3)
            m0["bounce"](47, h)
            m0["colexp"](0, 32)
            m1["quad"](1)
            for seg in SEGS_A:
                m0["store"](seg, nc.gpsimd)
            m0["colexp"](32, 64)
            m0["colexp"](64, h)
            m1["ptt_rem"]()
            m1["quad"](2)
            for seg in SEGS_B:
                m0["store"](seg, nc.gpsimd)
            m1["ptt_quad"](0)
            m1["bounce"](0, 15)
            m1["quad"](3)
            m1["ptt_quad"](1)
            m1["bounce"](15, 31)
            m1["ptt_quad"](2)
            m1["bounce"](31, 47)
            m1["ptt_quad"](3)
            m1["bounce"](47, h)
            m1["colexp"](0, 32)
            for seg in SEGS_A:
                m1["store"](seg, nc.sync)
            m1["colexp"](32, 64)
            m1["colexp"](64, h)
            tail_eng = [nc.sync, nc.scalar] * ((len(SEGS_B) + 1) // 2)
            for seg, eng in zip(SEGS_B, tail_eng):
                m1["store"](seg, eng)

    nc.compile()
    return nc


_cached_nc = None


def _get_nc():
    global _cached_nc
    if _cached_nc is None:
        _cached_nc = build_module()
    return _cached_nc


def make_in_maps(feature_s2t, feature_target, centroid_s2t, centroid_target):
    in_maps = []
    for b in range(B):
        in_maps.append(
            {
                "feature_s2t": np.ascontiguousarray(
                    feature_s2t[b], dtype=np.float32
                ).reshape(C, HW),
                "feature_target": np.ascontiguousarray(
                    feature_target[b], dtype=np.float32
                ).reshape(C, HW),
                "centroid_s2t": np.ascontiguousarray(centroid_s2t, dtype=np.float32),
                "centroid_target": np.ascontiguousarray(
                    centroid_target, dtype=np.float32
                ),
            }
        )
    return in_maps


def kernel(
    feature_s2t,
    feature_target,
    centroid_s2t,
    centroid_target,
    seg_s2t=None,
    seg_target=None,
    **_unused,
):
    from concourse.bass_utils import run_bass_kernel_spmd

    nc = _get_nc()
    in_maps = make_in_maps(
        np.asarray(feature_s2t),
        np.asarray(feature_target),
        np.asarray(centroid_s2t),
        np.asarray(centroid_target),
    )
    res = run_bass_kernel_spmd(nc, in_maps, core_ids=list(range(B)))
    results = res.results
    m0 = np.stack([results[b]["out0"] for b in range(B)]).astype(np.int32)
    m1 = np.stack([results[b]["out1"] for b in range(B)]).astype(np.int32)
    return (m0, m1)


# revision 39
# speedup vs baseline: 1.0683x; 1.0683x over previous
"""Trainium2 Bass kernel for nn_Cross_Domain_Class_Alignment.

Reference computation (per sample b):
    mask0[b] = argmin_k || feature_s2t[b,:,r,c] - centroid_target[k] ||^2
    mask1[b] = argmin_k || feature_target[b,:,r,c] - centroid_s2t[k] ||^2
    both nearest-upsampled from (65,129) to (512,1024), int32.

Sharding: data-parallel over batch B=8 across 8 NeuronCores (1 sample/core).
Centroids are replicated.

Per-core dataflow (per mask):
  - features [256, 8385] streamed in 2048-pixel slices x 2 channel chunks;
    the 193-px remainder is processed FIRST so the pipeline tail is short
  - dist matmuls, centroid-stationary: psum quad [128, 512] holds four
    512-pixel banks stacked at partition offsets {0,32,64,96} via
    tile_position col-tiling.  Two chunk matmuls accumulate C=256.
  - scalar-engine copy fuses m = 2*dots - csq (per-partition bias) while
    moving the quad PSUM->SBUF
  - ONE DVE stream-transpose (32x32 blocks) per quad flips classes onto
    columns; the DVE argmin (reduce_max / is_ge / *(19-k) / reduce_max,
    first-index tie-break) then reduces within blocks; the {0,1}/weight
    tail runs in bf16 (exact).  This keeps the PE off the critical path
    (fp32 PE transposes run at 2 cycles/row and were the tail bottleneck).
  - per quad: PE transpose of ptf [128,16] -> scalar ACT fuses
    idx = 19 - y and casts to int8 -> 8KB DRAM bounce (block-strided
    scratch AP) reshapes the flat pixel order into m [65, 129] int8
  - column nearest-upsample 129 -> 1024 in THREE DVE copies via the affine
    decomposition out[127a + 8c + k] = m[16a + c] (+ two edge arms)
  - row nearest-upsample is folded into the output DMAs: 16 row-segment
    stores with stride-0 (broadcast) source APs replicate each source row
    7-8x on the way to DRAM; output is int8 (cast to int32 on host)
"""

import numpy as np

B, C, h, w = 8, 256, 65, 129
K = 19
H, W = 512, 1024
HW = h * w              # 8385
QUAD_PX = 2048          # four 512-px banks per psum quad
NFULL = HW // QUAD_PX   # 4 full quads
REM = HW - NFULL * QUAD_PX   # 193 remainder pixels
NT = (HW + 127) // 128  # 66 pixel blocks of 128


def _row_segs():
    """Runs of equal repeat count in the row map ri[r'] = r'*65 // 512."""
    ri = (np.arange(H) * h) // H
    rreps = np.bincount(ri, minlength=h)
    segs, i, dst = [], 0, 0
    while i < h:
        j = i
        while j < h and rreps[j] == rreps[i]:
            j += 1
        segs.append((i, j - i, int(rreps[i]), dst))
        dst += (j - i) * int(rreps[i])
        i = j
    assert dst == H
    return segs


# stripe split at row 32: DVE access patterns must start at a partition
# multiple of 32, so the colexp stripes are [0,32) and [32,65)
ROW_SEGS = _row_segs()
SEGS_A = [s for s in ROW_SEGS if s[0] + s[1] - 1 <= 31]   # rows 0..31
SEGS_B = [s for s in ROW_SEGS if s[0] + s[1] - 1 > 31]    # rows 32..64


def build_module(num_devices=8):
    import concourse.bass as bass
    import concourse.tile as tile
    from concourse import bacc, mybir
    import ml_dtypes

    f32 = mybir.dt.float32
    f32r = mybir.dt.float32r
    bf16 = mybir.dt.bfloat16
    i8 = mybir.dt.int8

    nc = bacc.Bacc(
        "TRN2",
        target_bir_lowering=False,
        debug=False,
        enable_asserts=False,
        num_devices=num_devices,
    )

    f_s2t = nc.dram_tensor("feature_s2t", [C, HW], f32, kind="ExternalInput")
    f_tgt = nc.dram_tensor("feature_target", [C, HW], f32, kind="ExternalInput")
    c_s2t = nc.dram_tensor("centroid_s2t", [K, C], f32, kind="ExternalInput")
    c_tgt = nc.dram_tensor("centroid_target", [K, C], f32, kind="ExternalInput")
    out0 = nc.dram_tensor("out0", [H, W], i8, kind="ExternalOutput")
    out1 = nc.dram_tensor("out1", [H, W], i8, kind="ExternalOutput")

    ident_dram = nc.inline_tensor(np.eye(128, dtype=np.float32), name="ident_const")
    # sel[k, 32j+k] = -1.0: replicates -csq over the four 32-partition groups
    sel_np = np.zeros((K, 128), dtype=np.float32)
    for j in range(4):
        sel_np[np.arange(K), 32 * j + np.arange(K)] = -1.0
    sel_dram = nc.inline_tensor(sel_np, name="sel_const")
    wk_np = np.tile(
        (K - np.arange(K)).astype(ml_dtypes.bfloat16), (128, 1)
    )
    wk_dram = nc.inline_tensor(wk_np, name="wk_const")

    X = mybir.AxisListType.X
    ALU = mybir.AluOpType
    AF = mybir.ActivationFunctionType

    def strided(ap, off, dims):
        """Replace the free dims of `ap` with explicit (stride, size) pairs."""
        a = ap.copy()
        a.ap = a.ap[0:1] + [[s, n] for (s, n) in dims]
        a.offset = a.offset + off
        return a

    def dram_strided(ap, off, dims):
        """Replace ALL dims of a DRAM `ap` with explicit (stride, size) pairs."""
        a = ap.copy()
        a.ap = a.ap[0:0] + [[s, n] for (s, n) in dims]
        a.offset = a.offset + off
        return a

    with tile.TileContext(nc) as tc:
        from contextlib import ExitStack

        with ExitStack() as ctx:
            const_p = ctx.enter_context(tc.tile_pool(name="const", bufs=1))
            feat_p = ctx.enter_context(tc.tile_pool(name="feat", bufs=7))
            q_p = ctx.enter_context(tc.tile_pool(name="q", bufs=4))
            qt_p = ctx.enter_context(tc.tile_pool(name="qt", bufs=4))
            mx_p = ctx.enter_context(tc.tile_pool(name="mx", bufs=3))
            eq_p = ctx.enter_context(tc.tile_pool(name="eq", bufs=3))
            pt_p = ctx.enter_context(tc.tile_pool(name="pt", bufs=3))
            m_p = ctx.enter_context(tc.tile_pool(name="m", bufs=2))
            ps_dist = ctx.enter_context(tc.tile_pool(name="psd", bufs=4, space="PSUM"))
            ps_tr = ctx.enter_context(tc.tile_pool(name="pst", bufs=3, space="PSUM"))
            dram_p = ctx.enter_context(tc.tile_pool(name="dram", bufs=2, space="DRAM"))

            # ---- constants (centroids first: they gate the prep chain) ----
            cent_sbs = {}
            for pidx, cdram in ((0, c_tgt), (1, c_s2t)):
                cs = const_p.tile(
                    [K, C], f32, tag=f"cent{pidx}", name=f"cent_sb{pidx}"
                )
                nc.scalar.dma_start(out=cs[:], in_=cdram[:, :])
                cent_sbs[pidx] = cs
            ident = const_p.tile([128, 128], f32, tag="ident")
            nc.scalar.dma_start(out=ident[:], in_=ident_dram[:, :])
            wk_sb = const_p.tile([128, K], bf16, tag="wk")
            nc.scalar.dma_start(out=wk_sb[:], in_=wk_dram[:, :])
            sel_sb = const_p.tile([K, 128], f32, tag="sel")
            nc.scalar.dma_start(out=sel_sb[:], in_=sel_dram[:, :])

            # ---- per-pair centroid prep ----
            def prep_pair(pidx):
                cent_sb = cent_sbs[pidx]
                sq = const_p.tile([K, C], f32, tag=f"centsq{pidx}")
                nc.vector.tensor_mul(sq[:], cent_sb[:], cent_sb[:])
                csq = const_p.tile([K, 1], f32, tag=f"csq{pidx}")
                nc.vector.reduce_sum(csq[:], sq[:], axis=X)
                # -csq replicated at partition offsets {0,32,64,96}
                pb = ps_tr.tile([128, 1], f32, tag="tr")
                nc.tensor.matmul(pb[:], sel_sb[:], csq[:], start=True, stop=True)
                csqn4 = const_p.tile([128, 1], f32, tag=f"csqn4_{pidx}")
                nc.vector.tensor_copy(out=csqn4[:], in_=pb[:])
                # centT chunks [128, 32]: cols 0:19 = cent^T, cols 19:32 = 0
                centT = []
                for cc in range(2):
                    ct = const_p.tile([128, 32], f32, tag=f"centT{pidx}_{cc}")
                    nc.vector.memset(ct[:], 0.0)
                    pt = ps_tr.tile([128, K], f32, tag="tr")
                    nc.tensor.transpose(
                        pt[:], cent_sb[:, cc * 128 : (cc + 1) * 128], ident[:K, :K]
                    )
                    nc.vector.tensor_copy(out=ct[:, 0:K], in_=pt[:])
                    centT.append(ct)
                return centT, csqn4

            centT_tgt, csqn4_tgt = prep_pair(0)   # for mask0 (feature_s2t)
            centT_s2t, csqn4_s2t = prep_pair(1)   # for mask1 (feature_target)

            def stream_mask(feat, centT, csqn4, out_dram, tail_engines):
                # ptf: quad Bq at cols 16*Bq+n (partition 32j+c''), rem at
                # cols 64..72 (partitions 0..31); px = 2048*Bq + 512j + 32n + c''
                ptf = pt_p.tile([128, 72], f32, tag="ptf")
                msb = m_p.tile([h, w], i8, tag="m")
                e_sb = m_p.tile([h, W], i8, tag="e")
                scratch = dram_p.tile([NT, 128], i8, tag="scratch")
                scratch_flat = scratch[:].rearrange("a b -> (a b)")

                def load_range(px0, pxw):
                    fg = []
                    for cc in range(2):
                        ft = feat_p.tile([128, QUAD_PX], f32, tag=f"feat{cc}")
                        half = (pxw + 1) // 2
                        nc.sync.dma_start(
                            out=ft[:, 0:half],
                            in_=feat[cc * 128 : (cc + 1) * 128, px0 : px0 + half],
                        )
                        nc.sync.dma_start(
                            out=ft[:, half:pxw],
                            in_=feat[cc * 128 : (cc + 1) * 128, px0 + half : px0 + pxw],
                        )
                        fg.append(ft)
                    return fg

                def argmin_g(src, npart, g, out_ap):
                    # y = 19 - argmin over k, first-index tie-break; src is
                    # the stream-transposed quad, class k at col 32*n + k
                    view = strided(src, 0, [(32, g), (1, K)])
                    mxt = mx_p.tile([128, g], f32, tag="mx")
                    nc.vector.tensor_reduce(mxt[0:npart, :], view, axis=X, op=ALU.max)
                    eqt = eq_p.tile([128, g * K], bf16, tag="eq")
                    eqv = eqt[0:npart, :].rearrange("p (g k) -> p g k", k=K)
                    nc.vector.tensor_tensor(
                        out=eqv,
                        in0=view,
                        in1=mxt[0:npart, :].unsqueeze(2).broadcast_to([npart, g, K]),
                        op=ALU.is_ge,
                    )
                    nc.vector.tensor_tensor(
                        out=eqv,
                        in0=eqv,
                        in1=wk_sb[0:npart, :].unsqueeze(1).broadcast_to([npart, g, K]),
                        op=ALU.mult,
                    )
                    nc.vector.tensor_reduce(out_ap, eqv, axis=X, op=ALU.max)

                def do_quad(Bq):
                    fg = load_range(Bq * QUAD_PX, QUAD_PX)
                    psq = ps_dist.tile([128, 512], f32, tag="dist")
                    for j in range(4):
                        for cc in range(2):
                            nc.tensor.matmul(
                                psq[32 * j : 32 * j + 32, :],
                                centT[cc][:],
                                fg[cc][:, 512 * j : 512 * j + 512],
                                start=(cc == 0),
                                stop=(cc == 1),
                                tile_position=(0, 32 * j),
                            )
                    quad = q_p.tile([128, 512], f32, tag="quad")
                    nc.scalar.activation(
                        out=quad[:],
                        in_=psq[:],
                        func=AF.Identity,
                        bias=csqn4[:],
                        scale=2.0,
                    )
                    # 32x32 block transpose: qT[32j+c'', 32n+k''] =
                    # value(class k'', px 2048*Bq + 512j + 32n + c'')
                    qT = qt_p.tile([128, 512], f32, tag="qt")
                    nc.vector.transpose(qT[:], quad[:])
                    argmin_g(qT[:], 128, 16, ptf[:, 16 * Bq : 16 * Bq + 16])

                def do_rem():
                    # remainder: 193 px (blocks 64, 65); done FIRST
                    px0 = NFULL * QUAD_PX
                    pxw = HW - px0
                    fg = load_range(px0, pxw)
                    psr = ps_dist.tile([32, 256], f32, tag="dist")
                    nc.vector.memset(psr[:, pxw:256], 0.0)
                    for cc in range(2):
                        nc.tensor.matmul(
                            psr[0:32, 0:pxw],
                            centT[cc][:],
                            fg[cc][:, 0:pxw],
                            start=(cc == 0),
                            stop=(cc == 1),
                        )
                    st2 = q_p.tile([32, 256], f32, tag="quad")
                    nc.scalar.activation(
                        out=st2[:],
                        in_=psr[:],
                        func=AF.Identity,
                        bias=csqn4[0:32, :],
                        scale=2.0,
                    )
                    qT = qt_p.tile([32, 256], f32, tag="qt")
                    nc.vector.transpose(qT[:], st2[:])
                    argmin_g(qT[:], 32, 8, ptf[0:32, 64:72])

                def ptt_quad(Bq):
                    # ptf quad cols -> scratch in flat pixel order, int8
                    pstt = ps_tr.tile([16, 128], f32, tag="tr")
                    nc.tensor.transpose(
                        pstt[:], ptf[:, 16 * Bq : 16 * Bq + 16], ident[:]
                    )
                    pttsb = pt_p.tile([16, 128], i8, tag="pttsb")
                    nc.scalar.activation(
                        out=pttsb[:],
                        in_=pstt[:],
                        func=AF.Copy,
                        bias=float(K),
                        scale=-1.0,
                    )
                    # scratch[2048*Bq + 512j + 32n + c''] <- pttsb[n, 32j+c'']
                    nc.scalar.dma_start(
                        out=dram_strided(
                            scratch_flat, 2048 * Bq, [(32, 16), (512, 4), (1, 32)]
                        ),
                        in_=strided(pttsb[:], 0, [(32, 4), (1, 32)]),
                    )

                def ptt_rem():
                    pstt = ps_tr.tile([8, 32], f32, tag="tr")
                    nc.tensor.transpose(
                        pstt[:], ptf[0:32, 64:72], ident[:32, :32]
                    )
                    pttsb = pt_p.tile([8, 32], i8, tag="pttsb")
                    nc.scalar.activation(
                        out=pttsb[:],
                        in_=pstt[:],
                        func=AF.Copy,
                        bias=float(K),
                        scale=-1.0,
                    )
                    # scratch[8192 + 32n + c''] <- pttsb[n, c'']
                    nc.scalar.dma_start(
                        out=dram_strided(scratch_flat, 8192, [(32, 8), (1, 32)]),
                        in_=pttsb[:],
                    )

                def bounce(r0, r1, eng=None):
                    (eng or nc.gpsimd).dma_start(
                        out=msb[r0:r1, :],
                        in_=scratch_flat[r0 * w : r1 * w].rearrange(
                            "(r c) -> r c", c=w
                        ),
                    )

                def colexp(r0, r1):
                    # column upsample 129 -> 1024: affine 3-arm decomposition
                    nr = r1 - r0
                    nc.vector.tensor_copy(
                        out=e_sb[r0:r1, 0:8],
                        in_=msb[r0:r1, 0:1].broadcast_to([nr, 8]),
                    )
                    nc.vector.tensor_copy(
                        out=strided(e_sb[r0:r1, :], 8, [(127, 8), (8, 15), (1, 8)]),
                        in_=strided(msb[r0:r1, :], 1, [(16, 8), (1, 15), (0, 8)]),
                    )
                    nc.vector.tensor_copy(
                        out=strided(e_sb[r0:r1, :], 128, [(127, 8), (1, 7)]),
                        in_=strided(msb[r0:r1, :], 16, [(16, 8), (0, 7)]),
                    )

                def store_seg(seg, eng):
                    src0, ns, rep, dst0 = seg
                    eng.dma_start(
                        out=out_dram[dst0 : dst0 + ns * rep, :].rearrange(
                            "(s r) c -> s r c", r=rep
                        ),
                        in_=e_sb[src0 : src0 + ns, :]
                        .unsqueeze(1)
                        .broadcast_to([ns, rep, W]),
                    )

                return {
                    "rem": do_rem,
                    "quad": do_quad,
                    "ptt_quad": ptt_quad,
                    "ptt_rem": ptt_rem,
                    "bounce": bounce,
                    "colexp": colexp,
                    "store": store_seg,
                }

            # The two masks are interleaved: each ptt transpose is emitted
            # ~two stream pieces behind its argmin so the in-order PE ring
            # never stalls on the DVE chain, and mask0's finish work rides
            # inside mask1's stream.
            m0 = stream_mask(f_s2t, centT_tgt, csqn4_tgt, out0, None)
            m1 = stream_mask(f_tgt, centT_s2t, csqn4_s2t, out1, None)

            m0["rem"]()
            m0["quad"](0)
            m0["quad"](1)
            m0["ptt_rem"]()
            m0["quad"](2)
            m0["ptt_quad"](0)
            m0["bounce"](0, 15)
            m0["quad"](3)
            m0["ptt_quad"](1)
            m0["bounce"](15, 31)
            m1["rem"]()
            m0["ptt_quad"](2)
            m0["bounce"](31, 47)
            m1["quad"](0)
            m0["ptt_quad"](3)
            m0["bounce"](47, h)
            m0["colexp"](0, 32)
            m1["quad"](1)
            for seg in SEGS_A:
                m0["store"](seg, nc.gpsimd)
            m0["colexp"](32, 64)
            m0["colexp"](64, h)
            m1["ptt_rem"]()
            m1["quad"](2)
            for seg in SEGS_B:
                m0["store"](seg, nc.gpsimd)
            m1["ptt_quad"](0)
            m1["bounce"](0, 15)
            m1["quad"](3)
            m1["ptt_quad"](1)
            m1["bounce"](15, 31)
            m1["ptt_quad"](2)
            m1["bounce"](31, 47)
            m1["ptt_quad"](3)
            m1["bounce"](47, h)
            m1["colexp"](0, 32)
            for seg in SEGS_A:
                m1["store"](seg, nc.gpsimd)
            m1["colexp"](32, 64)
            m1["colexp"](64, h)
            tail_eng = [nc.sync, nc.scalar] * ((len(SEGS_B) + 1) // 2)
            for seg, eng in zip(SEGS_B, tail_eng):
                m1["store"](seg, eng)

    nc.compile()
    return nc


_cached_nc = None


def _get_nc():
    global _cached_nc
    if _cached_nc is None:
        _cached_nc = build_module()
    return _cached_nc


def make_in_maps(feature_s2t, feature_target, centroid_s2t, centroid_target):
    in_maps = []
    for b in range(B):
        in_maps.append(
            {
                "feature_s2t": np.ascontiguousarray(
                    feature_s2t[b], dtype=np.float32
                ).reshape(C, HW),
                "feature_target": np.ascontiguousarray(
                    feature_target[b], dtype=np.float32
                ).reshape(C, HW),
                "centroid_s2t": np.ascontiguousarray(centroid_s2t, dtype=np.float32),
                "centroid_target": np.ascontiguousarray(
                    centroid_target, dtype=np.float32
                ),
            }
        )
    return in_maps


def kernel(
    feature_s2t,
    feature_target,
    centroid_s2t,
    centroid_target,
    seg_s2t=None,
    seg_target=None,
    **_unused,
):
    from concourse.bass_utils import run_bass_kernel_spmd

    nc = _get_nc()
    in_maps = make_in_maps(
        np.asarray(feature_s2t),
        np.asarray(feature_target),
        np.asarray(centroid_s2t),
        np.asarray(centroid_target),
    )
    res = run_bass_kernel_spmd(nc, in_maps, core_ids=list(range(B)))
    results = res.results
    m0 = np.stack([results[b]["out0"] for b in range(B)]).astype(np.int32)
    m1 = np.stack([results[b]["out1"] for b in range(B)]).astype(np.int32)
    return (m0, m1)


# revision 40
# speedup vs baseline: 1.1157x; 1.0444x over previous
"""Trainium2 Bass kernel for nn_Cross_Domain_Class_Alignment.

Reference computation (per sample b):
    mask0[b] = argmin_k || feature_s2t[b,:,r,c] - centroid_target[k] ||^2
    mask1[b] = argmin_k || feature_target[b,:,r,c] - centroid_s2t[k] ||^2
    both nearest-upsampled from (65,129) to (512,1024), int32.

Sharding: data-parallel over batch B=8 across 8 NeuronCores (1 sample/core).
Centroids are replicated.

Per-core dataflow (per mask):
  - features [256, 8385] streamed in 2048-pixel slices x 2 channel chunks;
    the 193-px remainder is processed FIRST so the pipeline tail is short
  - dist matmuls, centroid-stationary: psum quad [128, 512] holds four
    512-pixel banks stacked at partition offsets {0,32,64,96} via
    tile_position col-tiling.  Two chunk matmuls accumulate C=256.
  - scalar-engine copy fuses m = 2*dots - csq (per-partition bias) while
    moving the quad PSUM->SBUF
  - ONE DVE stream-transpose (32x32 blocks) per quad flips classes onto
    columns; the DVE argmin (reduce_max / is_ge / *(19-k) / reduce_max,
    first-index tie-break) then reduces within blocks; the {0,1}/weight
    tail runs in bf16 (exact).  This keeps the PE off the critical path
    (fp32 PE transposes run at 2 cycles/row and were the tail bottleneck).
  - per quad: PE transpose of ptf [128,16] -> scalar ACT fuses
    idx = 19 - y and casts to int8 -> 8KB DRAM bounce (block-strided
    scratch AP) reshapes the flat pixel order into m [65, 129] int8
  - column nearest-upsample 129 -> 1024 in THREE DVE copies via the affine
    decomposition out[127a + 8c + k] = m[16a + c] (+ two edge arms)
  - row nearest-upsample is folded into the output DMAs: 16 row-segment
    stores with stride-0 (broadcast) source APs replicate each source row
    7-8x on the way to DRAM; output is int8 (cast to int32 on host)
"""

import numpy as np

B, C, h, w = 8, 256, 65, 129
K = 19
H, W = 512, 1024
HW = h * w              # 8385
QUAD_PX = 2048          # four 512-px banks per psum quad
NFULL = HW // QUAD_PX   # 4 full quads
REM = HW - NFULL * QUAD_PX   # 193 remainder pixels
NT = (HW + 127) // 128  # 66 pixel blocks of 128


def _row_segs():
    """Runs of equal repeat count in the row map ri[r'] = r'*65 // 512."""
    ri = (np.arange(H) * h) // H
    rreps = np.bincount(ri, minlength=h)
    segs, i, dst = [], 0, 0
    while i < h:
        j = i
        while j < h and rreps[j] == rreps[i]:
            j += 1
        segs.append((i, j - i, int(rreps[i]), dst))
        dst += (j - i) * int(rreps[i])
        i = j
    assert dst == H
    return segs


# stripe split at row 32: DVE access patterns must start at a partition
# multiple of 32, so the colexp stripes are [0,32) and [32,65)
ROW_SEGS = _row_segs()
SEGS_A = [s for s in ROW_SEGS if s[0] + s[1] - 1 <= 31]   # rows 0..31
SEGS_B = [s for s in ROW_SEGS if s[0] + s[1] - 1 > 31]    # rows 32..64


def build_module(num_devices=8):
    import concourse.bass as bass
    import concourse.tile as tile
    from concourse import bacc, mybir
    import ml_dtypes

    f32 = mybir.dt.float32
    f32r = mybir.dt.float32r
    bf16 = mybir.dt.bfloat16
    i8 = mybir.dt.int8

    nc = bacc.Bacc(
        "TRN2",
        target_bir_lowering=False,
        debug=False,
        enable_asserts=False,
        num_devices=num_devices,
    )

    f_s2t = nc.dram_tensor("feature_s2t", [C, HW], f32, kind="ExternalInput")
    f_tgt = nc.dram_tensor("feature_target", [C, HW], f32, kind="ExternalInput")
    c_s2t = nc.dram_tensor("centroid_s2t", [K, C], f32, kind="ExternalInput")
    c_tgt = nc.dram_tensor("centroid_target", [K, C], f32, kind="ExternalInput")
    out0 = nc.dram_tensor("out0", [H, W], i8, kind="ExternalOutput")
    out1 = nc.dram_tensor("out1", [H, W], i8, kind="ExternalOutput")

    ident_dram = nc.inline_tensor(np.eye(128, dtype=np.float32), name="ident_const")
    # sel[k, 32j+k] = -1.0: replicates -csq over the four 32-partition groups
    sel_np = np.zeros((K, 128), dtype=np.float32)
    for j in range(4):
        sel_np[np.arange(K), 32 * j + np.arange(K)] = -1.0
    sel_dram = nc.inline_tensor(sel_np, name="sel_const")
    wk_np = np.tile(
        (K - np.arange(K)).astype(ml_dtypes.bfloat16), (128, 1)
    )
    wk_dram = nc.inline_tensor(wk_np, name="wk_const")

    X = mybir.AxisListType.X
    ALU = mybir.AluOpType
    AF = mybir.ActivationFunctionType

    def strided(ap, off, dims):
        """Replace the free dims of `ap` with explicit (stride, size) pairs."""
        a = ap.copy()
        a.ap = a.ap[0:1] + [[s, n] for (s, n) in dims]
        a.offset = a.offset + off
        return a

    def dram_strided(ap, off, dims):
        """Replace ALL dims of a DRAM `ap` with explicit (stride, size) pairs."""
        a = ap.copy()
        a.ap = a.ap[0:0] + [[s, n] for (s, n) in dims]
        a.offset = a.offset + off
        return a

    with tile.TileContext(nc) as tc:
        from contextlib import ExitStack

        with ExitStack() as ctx:
            const_p = ctx.enter_context(tc.tile_pool(name="const", bufs=1))
            feat_p = ctx.enter_context(tc.tile_pool(name="feat", bufs=7))
            q_p = ctx.enter_context(tc.tile_pool(name="q", bufs=4))
            qt_p = ctx.enter_context(tc.tile_pool(name="qt", bufs=4))
            mx_p = ctx.enter_context(tc.tile_pool(name="mx", bufs=3))
            eq_p = ctx.enter_context(tc.tile_pool(name="eq", bufs=3))
            pt_p = ctx.enter_context(tc.tile_pool(name="pt", bufs=3))
            m_p = ctx.enter_context(tc.tile_pool(name="m", bufs=2))
            ps_dist = ctx.enter_context(tc.tile_pool(name="psd", bufs=4, space="PSUM"))
            ps_tr = ctx.enter_context(tc.tile_pool(name="pst", bufs=3, space="PSUM"))
            dram_p = ctx.enter_context(tc.tile_pool(name="dram", bufs=2, space="DRAM"))

            # ---- constants (centroids first: they gate the prep chain) ----
            cent_sbs = {}
            for pidx, cdram in ((0, c_tgt), (1, c_s2t)):
                cs = const_p.tile(
                    [K, C], f32, tag=f"cent{pidx}", name=f"cent_sb{pidx}"
                )
                nc.scalar.dma_start(out=cs[:], in_=cdram[:, :])
                cent_sbs[pidx] = cs
            ident = const_p.tile([128, 128], f32, tag="ident")
            nc.scalar.dma_start(out=ident[:], in_=ident_dram[:, :])
            wk_sb = const_p.tile([128, K], bf16, tag="wk")
            nc.scalar.dma_start(out=wk_sb[:], in_=wk_dram[:, :])
            sel_sb = const_p.tile([K, 128], f32, tag="sel")
            nc.scalar.dma_start(out=sel_sb[:], in_=sel_dram[:, :])

            # ---- per-pair centroid prep ----
            def prep_pair(pidx):
                cent_sb = cent_sbs[pidx]
                sq = const_p.tile([K, C], f32, tag=f"centsq{pidx}")
                nc.vector.tensor_mul(sq[:], cent_sb[:], cent_sb[:])
                csq = const_p.tile([K, 1], f32, tag=f"csq{pidx}")
                nc.vector.reduce_sum(csq[:], sq[:], axis=X)
                # -csq replicated at partition offsets {0,32,64,96}
                pb = ps_tr.tile([128, 1], f32, tag="tr")
                nc.tensor.matmul(pb[:], sel_sb[:], csq[:], start=True, stop=True)
                csqn4 = const_p.tile([128, 1], f32, tag=f"csqn4_{pidx}")
                nc.vector.tensor_copy(out=csqn4[:], in_=pb[:])
                # centT chunks [128, 32]: cols 0:19 = cent^T, cols 19:32 = 0
                centT = []
                for cc in range(2):
                    ct = const_p.tile([128, 32], f32, tag=f"centT{pidx}_{cc}")
                    nc.vector.memset(ct[:], 0.0)
                    pt = ps_tr.tile([128, K], f32, tag="tr")
                    nc.tensor.transpose(
                        pt[:], cent_sb[:, cc * 128 : (cc + 1) * 128], ident[:K, :K]
                    )
                    nc.vector.tensor_copy(out=ct[:, 0:K], in_=pt[:])
                    centT.append(ct)
                return centT, csqn4

            centT_tgt, csqn4_tgt = prep_pair(0)   # for mask0 (feature_s2t)
            centT_s2t, csqn4_s2t = prep_pair(1)   # for mask1 (feature_target)

            def stream_mask(feat, centT, csqn4, out_dram, tail_engines):
                # ptf: quad Bq at cols 16*Bq+n (partition 32j+c''), rem at
                # cols 64..72 (partitions 0..31); px = 2048*Bq + 512j + 32n + c''
                ptf = pt_p.tile([128, 72], f32, tag="ptf")
                msb = m_p.tile([h, w], i8, tag="m")
                e_sb = m_p.tile([h, W], i8, tag="e")
                scratch = dram_p.tile([NT, 128], i8, tag="scratch")
                scratch_flat = scratch[:].rearrange("a b -> (a b)")

                def load_range(px0, pxw):
                    fg = []
                    for cc in range(2):
                        ft = feat_p.tile([128, QUAD_PX], f32, tag=f"feat{cc}")
                        half = (pxw + 1) // 2
                        nc.sync.dma_start(
                            out=ft[:, 0:half],
                            in_=feat[cc * 128 : (cc + 1) * 128, px0 : px0 + half],
                        )
                        nc.sync.dma_start(
                            out=ft[:, half:pxw],
                            in_=feat[cc * 128 : (cc + 1) * 128, px0 + half : px0 + pxw],
                        )
                        fg.append(ft)
                    return fg

                def argmin_g(src, npart, g, out_ap):
                    # y = 19 - argmin over k, first-index tie-break; src is
                    # the stream-transposed quad, class k at col 32*n + k
                    view = strided(src, 0, [(32, g), (1, K)])
                    mxt = mx_p.tile([128, g], f32, tag="mx")
                    nc.vector.tensor_reduce(mxt[0:npart, :], view, axis=X, op=ALU.max)
                    eqt = eq_p.tile([128, g * K], bf16, tag="eq")
                    eqv = eqt[0:npart, :].rearrange("p (g k) -> p g k", k=K)
                    nc.vector.tensor_tensor(
                        out=eqv,
                        in0=view,
                        in1=mxt[0:npart, :].unsqueeze(2).broadcast_to([npart, g, K]),
                        op=ALU.is_ge,
                    )
                    nc.vector.tensor_tensor(
                        out=eqv,
                        in0=eqv,
                        in1=wk_sb[0:npart, :].unsqueeze(1).broadcast_to([npart, g, K]),
                        op=ALU.mult,
                    )
                    nc.vector.tensor_reduce(out_ap, eqv, axis=X, op=ALU.max)

                def do_quad(Bq):
                    fg = load_range(Bq * QUAD_PX, QUAD_PX)
                    psq = ps_dist.tile([128, 512], f32, tag="dist")
                    for j in range(4):
                        for cc in range(2):
                            nc.tensor.matmul(
                                psq[32 * j : 32 * j + 32, :],
                                centT[cc][:],
                                fg[cc][:, 512 * j : 512 * j + 512],
                                start=(cc == 0),
                                stop=(cc == 1),
                                tile_position=(0, 32 * j),
                            )
                    quad = q_p.tile([128, 512], f32, tag="quad")
                    nc.scalar.activation(
                        out=quad[:],
                        in_=psq[:],
                        func=AF.Identity,
                        bias=csqn4[:],
                        scale=2.0,
                    )
                    # 32x32 block transpose: qT[32j+c'', 32n+k''] =
                    # value(class k'', px 2048*Bq + 512j + 32n + c'')
                    qT = qt_p.tile([128, 512], f32, tag="qt")
                    nc.vector.transpose(qT[:], quad[:])
                    argmin_g(qT[:], 128, 16, ptf[:, 16 * Bq : 16 * Bq + 16])

                def do_rem():
                    # remainder: 193 px (blocks 64, 65); done FIRST
                    px0 = NFULL * QUAD_PX
                    pxw = HW - px0
                    fg = load_range(px0, pxw)
                    psr = ps_dist.tile([32, 256], f32, tag="dist")
                    nc.vector.memset(psr[:, pxw:256], 0.0)
                    for cc in range(2):
                        nc.tensor.matmul(
                            psr[0:32, 0:pxw],
                            centT[cc][:],
                            fg[cc][:, 0:pxw],
                            start=(cc == 0),
                            stop=(cc == 1),
                        )
                    st2 = q_p.tile([32, 256], f32, tag="quad")
                    nc.scalar.activation(
                        out=st2[:],
                        in_=psr[:],
                        func=AF.Identity,
                        bias=csqn4[0:32, :],
                        scale=2.0,
                    )
                    qT = qt_p.tile([32, 256], f32, tag="qt")
                    nc.vector.transpose(qT[:], st2[:])
                    argmin_g(qT[:], 32, 8, ptf[0:32, 64:72])

                def ptt_quad(Bq):
                    # ptf quad cols -> scratch in flat pixel order, int8
                    pstt = ps_tr.tile([16, 128], f32, tag="tr")
                    nc.tensor.transpose(
                        pstt[:], ptf[:, 16 * Bq : 16 * Bq + 16], ident[:]
                    )
                    pttsb = pt_p.tile([16, 128], i8, tag="pttsb")
                    nc.scalar.activation(
                        out=pttsb[:],
                        in_=pstt[:],
                        func=AF.Copy,
                        bias=float(K),
                        scale=-1.0,
                    )
                    # scratch[2048*Bq + 512j + 32n + c''] <- pttsb[n, 32j+c'']
                    nc.scalar.dma_start(
                        out=dram_strided(
                            scratch_flat, 2048 * Bq, [(32, 16), (512, 4), (1, 32)]
                        ),
                        in_=strided(pttsb[:], 0, [(32, 4), (1, 32)]),
                    )

                def ptt_rem():
                    pstt = ps_tr.tile([8, 32], f32, tag="tr")
                    nc.tensor.transpose(
                        pstt[:], ptf[0:32, 64:72], ident[:32, :32]
                    )
                    pttsb = pt_p.tile([8, 32], i8, tag="pttsb")
                    nc.scalar.activation(
                        out=pttsb[:],
                        in_=pstt[:],
                        func=AF.Copy,
                        bias=float(K),
                        scale=-1.0,
                    )
                    # scratch[8192 + 32n + c''] <- pttsb[n, c'']
                    nc.scalar.dma_start(
                        out=dram_strided(scratch_flat, 8192, [(32, 8), (1, 32)]),
                        in_=pttsb[:],
                    )

                def bounce(r0, r1, eng=None):
                    (eng or nc.gpsimd).dma_start(
                        out=msb[r0:r1, :],
                        in_=scratch_flat[r0 * w : r1 * w].rearrange(
                            "(r c) -> r c", c=w
                        ),
                    )

                def colexp(r0, r1):
                    # column upsample 129 -> 1024: affine 3-arm decomposition
                    nr = r1 - r0
                    nc.vector.tensor_copy(
                        out=e_sb[r0:r1, 0:8],
                        in_=msb[r0:r1, 0:1].broadcast_to([nr, 8]),
                    )
                    nc.vector.tensor_copy(
                        out=strided(e_sb[r0:r1, :], 8, [(127, 8), (8, 15), (1, 8)]),
                        in_=strided(msb[r0:r1, :], 1, [(16, 8), (1, 15), (0, 8)]),
                    )
                    nc.vector.tensor_copy(
                        out=strided(e_sb[r0:r1, :], 128, [(127, 8), (1, 7)]),
                        in_=strided(msb[r0:r1, :], 16, [(16, 8), (0, 7)]),
                    )

                def store_seg(seg, eng):
                    src0, ns, rep, dst0 = seg
                    eng.dma_start(
                        out=out_dram[dst0 : dst0 + ns * rep, :].rearrange(
                            "(s r) c -> s r c", r=rep
                        ),
                        in_=e_sb[src0 : src0 + ns, :]
                        .unsqueeze(1)
                        .broadcast_to([ns, rep, W]),
                    )

                return {
                    "rem": do_rem,
                    "quad": do_quad,
                    "ptt_quad": ptt_quad,
                    "ptt_rem": ptt_rem,
                    "bounce": bounce,
                    "colexp": colexp,
                    "store": store_seg,
                }

            # The two masks are interleaved: each ptt transpose is emitted
            # ~two stream pieces behind its argmin so the in-order PE ring
            # never stalls on the DVE chain, and mask0's finish work rides
            # inside mask1's stream.
            m0 = stream_mask(f_s2t, centT_tgt, csqn4_tgt, out0, None)
            m1 = stream_mask(f_tgt, centT_s2t, csqn4_s2t, out1, None)

            m0["rem"]()
            m0["quad"](0)
            m0["quad"](1)
            m0["ptt_rem"]()
            m0["quad"](2)
            m0["ptt_quad"](0)
            m0["bounce"](0, 15)
            m0["quad"](3)
            m0["ptt_quad"](1)
            m0["bounce"](15, 31)
            m1["rem"]()
            m0["ptt_quad"](2)
            m0["bounce"](31, 47)
            m1["quad"](0)
            m0["ptt_quad"](3)
            m0["bounce"](47, h)
            m0["colexp"](0, 32)
            m1["quad"](1)
            for seg in SEGS_A:
                m0["store"](seg, nc.gpsimd)
            m0["colexp"](32, 64)
            m0["colexp"](64, h)
            m1["ptt_rem"]()
            m1["quad"](2)
            for seg in SEGS_B:
                m0["store"](seg, nc.gpsimd)
            m1["ptt_quad"](0)
            m1["bounce"](0, 15)
            m1["ptt_quad"](1)
            m1["bounce"](15, 31)
            m1["quad"](3)
            m1["ptt_quad"](2)
            m1["bounce"](31, 47)
            m1["ptt_quad"](3)
            m1["bounce"](47, h)
            m1["colexp"](0, 32)
            for seg in SEGS_A:
                m1["store"](seg, nc.gpsimd)
            m1["colexp"](32, 64)
            m1["colexp"](64, h)
            tail_eng = [nc.sync, nc.scalar] * ((len(SEGS_B) + 1) // 2)
            for seg, eng in zip(SEGS_B, tail_eng):
                m1["store"](seg, eng)

    nc.compile()
    return nc


_cached_nc = None


def _get_nc():
    global _cached_nc
    if _cached_nc is None:
        _cached_nc = build_module()
    return _cached_nc


def make_in_maps(feature_s2t, feature_target, centroid_s2t, centroid_target):
    in_maps = []
    for b in range(B):
        in_maps.append(
            {
                "feature_s2t": np.ascontiguousarray(
                    feature_s2t[b], dtype=np.float32
                ).reshape(C, HW),
                "feature_target": np.ascontiguousarray(
                    feature_target[b], dtype=np.float32
                ).reshape(C, HW),
                "centroid_s2t": np.ascontiguousarray(centroid_s2t, dtype=np.float32),
                "centroid_target": np.ascontiguousarray(
                    centroid_target, dtype=np.float32
                ),
            }
        )
    return in_maps


def kernel(
    feature_s2t,
    feature_target,
    centroid_s2t,
    centroid_target,
    seg_s2t=None,
    seg_target=None,
    **_unused,
):
    from concourse.bass_utils import run_bass_kernel_spmd

    nc = _get_nc()
    in_maps = make_in_maps(
        np.asarray(feature_s2t),
        np.asarray(feature_target),
        np.asarray(centroid_s2t),
        np.asarray(centroid_target),
    )
    res = run_bass_kernel_spmd(nc, in_maps, core_ids=list(range(B)))
    results = res.results
    m0 = np.stack([results[b]["out0"] for b in range(B)]).astype(np.int32)
    m1 = np.stack([results[b]["out1"] for b in range(B)]).astype(np.int32)
    return (m0, m1)
